# revision 83
# baseline (speedup 1.0000x reference)
"""Trainium2 Bass kernel for nn_Chambers (6-tower MLP + coupled sigmoid recurrence).

Data-parallel over 8 NeuronCores: each core processes a 16384-sample shard in
16 chunks of 1024 samples. res tiles are PE-transposed (fp32, exact) into
[100, 1024] activation tiles via rotation PSUM tiles; the 4 MLP layers run as
fp32r matmuls (full PE rate at N=512) with chamber pairs packed into 128
partition rows via shifted zero-padded stationaries. L4 lands in a per-chunk
[6, 1024] rotation psum tile that the (otherwise idle) Pool engine copies
into the [96, 512] raw half-tiles, so no ACT instruction ever touches raw.
The 5-step coupled sigmoid recurrence runs on the two independent 512-col
halves with ping-pong act tiles, so the chains never serialize.

Sync discipline: at most 1 sem wait + 1 update per engine instruction.
Cross-engine deps are pre-observed by real "touch" ops (tiny matmuls into
virgin ps_scr cells on PE; 1-elem copies on ACT/DVE/Pool), which establishes
the engine-clock coverage the sem-assignment pass needs to elide duplicate
waits; each rotation tile's first matmul then carries only the unavoidable
psum-drain WAW wait. PSUM budget: 3 rotation tags x [128,1024] (6 banks) +
ps_scr (1) + 1 spare = 8 banks.
"""
import numpy as np

import concourse.bass as bass
import concourse.mybir as mybir
from concourse.bass_utils import run_bass_kernel_spmd
from concourse.tile import TileContext
from concourse.tile_scheduler import N_PROCS
from concourse.vector_clock import ScopedClock
from bass_rust import add_dep_helper

# All gpsimd (SWDGE) DMAs share one completion-sem lane so consumers of the
# DMA-assembled raw tile carry a single wait.
import concourse.tile_sem_assignment as _tsa
if not getattr(_tsa.TileClockTick, "_single_swdge", False):
    _orig_tick_init = _tsa.TileClockTick.__init__

    def _tick_init(self, *a, **kw):
        _orig_tick_init(self, *a, **kw)
        self.swdge_sem_count = 1

    _tsa.TileClockTick.__init__ = _tick_init
    _tsa.TileClockTick._single_swdge = True

F32 = mybir.dt.float32
F32R = mybir.dt.float32r
AF = mybir.ActivationFunctionType
ALU = mybir.AluOpType

B = 131072
NCORES = 8
BS = B // NCORES           # 16384 samples per core
T = 1024                   # chunk (samples)
NCH = BS // T              # 16 chunks
RES_DIM = 100
CF_ITERS = 5
CF_K = 0.02

# wf (fp32) column layout
IDC = 0        # identity [128,128] for PE transposes
B1C = 128      # 6 cols: b1 per chamber
B2PC = 134     # 3 cols: pair-packed b2 (rows 0:64 even, 64:128 odd)
B3PC = 137     # 1 col: merged pairs 0,1 b3 (rows 32c..32c+32 = b3[c])
B3P2 = 138     # 1 col: pair2 b3 (rows 0:32 b3[4], 32:64 b3[5])
B4C = 139      # 1 col: b4 tiled x16 over 96 rows
FCOLS = 140

# wr (fp32r) column layout
W1C = 0                    # 6*128
W2EC = 768                 # 3*64  (even chambers)
W2OC = 960                 # 3*128 (odd chambers shifted to out rows 64:127)
W3AC = 1344                # 128 (pair1 shifted to out rows 64:127)
W3BC0 = 1472               # 64  (pair0 -> out rows 0:63)
W3P2C = 1536               # 64  (pair2 -> out rows 0:63)
W4AC = 1600                # 6 (chambers 0-3 from h3a rows 32c..)
W4BC = 1606                # 6 (chambers 4,5 from h3b rows 0:64)
CDC = 1792                 # 96 (block-diag decay*coupling*k per 6-row group)
I96C = 1888                # 96 identity
RCOLS = 1984


class TC(TileContext):
    """TileContext with a walrus-compatible epilogue (split final waits)."""

    def _drain_and_barrier(self, tick_clock, wait_clock):
        nc = self.nc
        full = ScopedClock({None: tick_clock.global_clock})
        for scope, vc in full.items():
            for proc in range(N_PROCS):
                t = vc.peek_next(proc) - 1
                if t > 0:
                    sc = ScopedClock()
                    sc.require_at_least(scope, proc, t)
                    w = nc.sync.nop(nofuse=True)
                    wait_clock.add_sem_waits(w.ins, sc)
        for eng in nc.engines.values():
            eng.drain(fusable=False)
        nc.all_engine_barrier(sem_only=True)
        assert self.sems is not None
        popped = nc._tile_sem_poison_stack.pop()
        assert popped is self._sem_poison
        nc.clear_and_free_semaphores(list(self.sems.allocated().values()))
        for eng in nc.engines.values():
            eng.drain(fusable=False)
        nc.all_engine_barrier(sem_only=True)


def _order(after_inst, before_inst):
    if after_inst is not None and before_inst is not None:
        add_dep_helper(after_inst.ins, before_inst.ins, sync=False, reason="order")


def build_module():
    nc = bass.Bass()
    res_d = nc.dram_tensor("res", [BS, RES_DIM], F32, kind="ExternalInput")
    wf_d = nc.dram_tensor("wf", [128, FCOLS], F32, kind="ExternalInput")
    wr_d = nc.dram_tensor("wr", [128, RCOLS], F32R, kind="ExternalInput")
    raw_d = nc.dram_tensor("raw_out", [96, T], F32, kind="ExternalOutput")
    act_d = nc.dram_tensor("act_out", [96, T], F32, kind="ExternalOutput")

    with TC(nc) as tc:
        with (
            tc.tile_pool(name="wconst", bufs=1) as wpool,
            tc.tile_pool(name="sbrt", bufs=3) as sbrt,
            tc.tile_pool(name="sbh", bufs=2) as sbh,
            tc.tile_pool(name="sbrec", bufs=1) as sbrec,
            tc.tile_pool(name="psscr", bufs=1, space="PSUM") as psscr,
            tc.tile_pool(name="pstr", bufs=1, space="PSUM") as pstr,
            tc.tile_pool(name="psmm", bufs=3, space="PSUM") as psmm,
        ):
            # DMA issue order: chunk-0 res and L1 weights first so compute
            # starts early; the bulky remainder streams behind.
            res_sb0a = wpool.tile([128, 4 * RES_DIM], F32)
            nc.sync.dma_start(
                out=res_sb0a[:],
                in_=res_d[0:T // 2].rearrange("(p n) d -> p (n d)", p=128))
            wf = wpool.tile([128, FCOLS], F32)
            nc.sync.dma_start(out=wf[:], in_=wf_d[:])
            res_sb0b = wpool.tile([128, 4 * RES_DIM], F32)
            nc.sync.dma_start(
                out=res_sb0b[:],
                in_=res_d[T // 2:T].rearrange("(p n) d -> p (n d)", p=128))
            wr = wpool.tile([128, RCOLS], F32R)
            nc.sync.dma_start(out=wr[:, 0:768], in_=wr_d[:, 0:768])
            # Bulk loads go through SWDGE (gpsimd) so the 8 HWDGE lanes
            # stay single-use (4 inputs + 4 output DMAs). Gate the SWDGE
            # triggers on the last startup HWDGE load so the bulk transfers
            # don't jump ahead of the critical-path ones on the shared DMA
            # engines.
            res_sb1 = wpool.tile([128, 3 * 8 * RES_DIM], F32)
            res_sb2 = wpool.tile([128, (NCH - 4) * 8 * RES_DIM], F32)
            ident = wf[:, IDC:IDC + 128]

            # Recurrence state: independent s=0/1 half tiles; ping-pong act
            # tiles so adjacent iterations never rewrite a tile the other
            # chain still reads (tile-granular deps would serialize).
            raw_sb = [sbrec.tile([96, 512], F32, name=f"raw_sb{s}")
                      for s in range(2)]
            act_pp = [[sbrec.tile([96, 512], F32R, name=f"act_p{s}{q}")
                       for q in range(2)] for s in range(2)]
            act_oh = [sbrec.tile([96, 512], F32, name=f"act_o{s}")
                      for s in range(2)]
            raw_rh = [sbrec.tile([96, 512], F32R, name=f"raw_r{s}")
                      for s in range(2)]
            scr = sbrec.tile([1, 4], F32)
            scrA2 = sbrec.tile([96, 16], F32)
            scrP2 = sbrec.tile([1, 16], F32)
            scrA = sbrec.tile([1, 160], F32)
            scrD = sbrec.tile([1, 64], F32)
            scrP = sbrec.tile([1, 64], F32)

            ps_scr = psscr.tile([128, 512], F32)

            # Bulk SWDGE loads, gated (via Pool) on the last startup HWDGE
            # load so they don't jump ahead on the shared DMA engines.
            gate = nc.gpsimd.tensor_copy(scrP[0:1, 63:64], wr[0:1, 0:1])
            d1 = nc.gpsimd.dma_start(out=wr[:, 768:], in_=wr_d[:, 768:])
            _order(d1, gate)
            d2 = nc.gpsimd.dma_start(
                out=res_sb1[:],
                in_=res_d[T:4 * T].rearrange("(p n) d -> p (n d)", p=128))
            _order(d2, d1)
            d3 = nc.gpsimd.dma_start(
                out=res_sb2[:],
                in_=res_d[4 * T:].rearrange("(p n) d -> p (n d)", p=128))
            _order(d3, d2)
            pool_tail0 = d3

            # PE warm-up touch: observe the wf DMA lane (identity) first;
            # the wr lane is observed later, right before L1 needs it, so
            # the chunk-0 transposes are not blocked behind it.
            warm_f = nc.tensor.matmul(ps_scr[0:1, 498:500], wf[0:1, 0:1],
                                      wf[0:1, 0:2], start=True, stop=True)
            # ACT touch: observe the wf lane before first bias use
            nc.scalar.activation(scr[0:1, 0:1], wf[0:1, B1C:B1C + 1], AF.Copy)

            pe_tail = warm_f
            act_tail = None
            dve_tail = None
            pool_tail = pool_tail0
            tcol = [0]
            acol = [0]
            dcol = [0]
            pcol = [0]

            def pe_touch(src_ap):
                nonlocal pe_tail
                t = tcol[0]; tcol[0] += 1
                assert t < 3 * 246
                row = 32 * (t // 246)
                col = 2 * (t % 246)
                m = nc.tensor.matmul(ps_scr[row:row + 1, col:col + 2],
                                     src_ap[:, 0:1], src_ap[:, 0:2],
                                     start=True, stop=True)
                _order(m, pe_tail)
                pe_tail = m
                return m

            def act_touch(src_ap):
                nonlocal act_tail
                t = acol[0]; acol[0] += 1
                assert t < 160
                s = nc.scalar.activation(scrA[0:1, t:t + 1], src_ap, AF.Copy)
                _order(s, act_tail)
                act_tail = s
                return s

            def dve_touch(src_ap):
                nonlocal dve_tail
                t = dcol[0]; dcol[0] += 1
                assert t < 64
                c = nc.vector.tensor_copy(scrD[0:1, t:t + 1], src_ap)
                _order(c, dve_tail)
                dve_tail = c
                return c

            def pool_touch(src_ap):
                nonlocal pool_tail
                t = pcol[0]; pcol[0] += 1
                assert t < 64
                c = nc.gpsimd.tensor_copy(scrP[0:1, t:t + 1], src_ap)
                _order(c, pool_tail)
                pool_tail = c
                return c

            tag_rr = [0]
            tag_state = [None, None, None]

            def new_mm_tile(name, touch=True, width=T):
                tg = tag_rr[0] % 3
                tag_rr[0] += 1
                st = tag_state[tg]
                if st is not None:
                    if touch:
                        tile_, row_, col_ = st
                        pe_touch(tile_[row_:row_ + 1, col_:col_ + 2])
                    tag_state[tg] = None
                t = psmm.tile([128, width], F32, tag=f"mm{tg}", bufs=1,
                              name=name)
                return t, tg

            def mm(out_ap, lhs_ap, rhs_ap, **kw):
                nonlocal pe_tail
                m = nc.tensor.matmul(out_ap, lhs_ap, rhs_ap, **kw)
                _order(m, pe_tail)
                pe_tail = m
                return m

            def silu(out_ap, pm_ap, bias_ap, out_tile, tg, func=AF.Silu):
                nonlocal act_tail
                s = nc.scalar.activation(out_ap, pm_ap, func, bias=bias_ap)
                _order(s, act_tail)
                act_tail = s
                if tg is not None:
                    tag_state[tg] = (out_tile, 0, 0)
                return s

            rt_tiles = {}
            h3_tiles = {}
            pstr_state = [None]  # cell observing the pstr bank's last release

            def pstr_tile(shape, release_src):
                """Allocate the shared pstr-bank tile; pre-observe the
                previous occupant's releasing instruction via its output
                cell, then record this tile's future release source."""
                if pstr_state[0] is not None:
                    pe_touch(pstr_state[0])
                t = pstr.tile(shape, F32, tag="tr", name="pstile")
                pstr_state[0] = release_src
                return t

            def emit_tr_half(i, h):
                """Transpose half h of chunk i's res into a [100, 512]
                f32r SBUF tile via the shared pstr psum bank."""
                nonlocal pe_tail, dve_tail
                if h == 0:
                    if i in (1, 4):
                        # observe the incoming res segment's DMA lane
                        rq0 = res_sb1 if i == 1 else res_sb2
                        pe_touch(rq0[0:1, 0:2])
                    rt_tiles[i] = []
                if i == 0:
                    pe_touch((res_sb0a, res_sb0b)[h][0:1, 0:2])
                rT = sbrt.tile([100, 512], F32R, tag="rT", bufs=6, name="rT")
                ptr = pstr_tile([100, 512], rT[0:1, 0:2])
                for n in range(4):
                    nn = 4 * h + n
                    if i == 0:
                        rq, coff = (res_sb0a, res_sb0b)[h], n * RES_DIM
                    elif i < 4:
                        rq, coff = res_sb1, ((i - 1) * 8 + nn) * RES_DIM
                    else:
                        rq, coff = res_sb2, ((i - 4) * 8 + nn) * RES_DIM
                    t_ = nc.tensor.transpose(
                        ptr[:, n * 128:(n + 1) * 128],
                        rq[:, coff:coff + RES_DIM],
                        ident,
                    )
                    _order(t_, pe_tail)
                    pe_tail = t_
                dve_touch(ptr[0:1, 0:1])
                cp = nc.vector.tensor_copy(rT[:], ptr[:])
                _order(cp, dve_tail)
                dve_tail = cp
                rt_tiles[i].append(rT)

            def emit_l4(j):
                """L4 for chunk j: rotation tile pm4[0:6] = W4a.h3a+W4b.h3b;
                ACT applies +b4 into a [6,T] staging tile and two SWDGE DMAs
                scatter it into the raw half-tiles rows 6j..6j+6 (the
                baseline-proven assembly path)."""
                nonlocal act_tail
                h3a, h3b = h3_tiles.pop(j)
                pe_touch(h3b[0:1, 0:2])   # h3 silus retired by now
                pm4, tg4 = new_mm_tile("pm4")
                for s in range(2):
                    mm(pm4[0:6, s * 512:(s + 1) * 512],
                       wr[:, W4AC:W4AC + 6],
                       h3a[:, s * 512:(s + 1) * 512],
                       start=True, stop=False)
                    mm(pm4[0:6, s * 512:(s + 1) * 512],
                       wr[0:64, W4BC:W4BC + 6],
                       h3b[0:64, s * 512:(s + 1) * 512],
                       start=False, stop=True)
                act_touch(pm4[0:1, 512:513])
                raw_i = sbh.tile([6, T], F32, tag="rawi", bufs=2,
                                 name="raw_i")
                ro = nc.scalar.activation(raw_i[:], pm4[0:6, :], AF.Identity,
                                          bias=wf[0:6, B4C:B4C + 1])
                _order(ro, act_tail)
                act_tail = ro
                tag_state[tg4] = (raw_i, 0, 0)
                # ACT observes the assembly DMAs (covers the raw_i slot WAR
                # two chunks later); Pool observes ACT through it
                ji = j % 16
                s_ = nc.scalar.activation(scrA2[:, ji:ji + 1],
                                          raw_sb[0][0:96, 0:1], AF.Copy)
                _order(s_, act_tail)
                act_tail = s_
                nc.gpsimd.tensor_copy(scrP2[0:1, ji:ji + 1],
                                      scrA2[0:1, ji:ji + 1])
                for s in range(2):
                    nc.gpsimd.dma_start(
                        out=raw_sb[s][6 * j:6 * j + 6, :],
                        in_=raw_i[:, s * 512:(s + 1) * 512])

            emit_tr_half(0, 0)
            emit_tr_half(0, 1)
            for i in range(NCH):
                rTs = rt_tiles.pop(i)

                # L1: 3 chamber-pairs
                h1s = []
                for cp in range(3):
                    ha = sbh.tile([128, T], F32R, tag="h1", bufs=7, name="h1a")
                    hb = sbh.tile([128, T], F32R, tag="h1", bufs=7, name="h1b")
                    pa, ta = new_mm_tile("pm1a", touch=False)
                    if i == 0 and cp == 0:
                        pe_touch(wr[0:1, 0:2])   # wr head-segment lane
                        # chunk-0 warm-up: keep PE busy across the DVE rT
                        # copy so the p-state ramp doesn't reset; outputs
                        # land in the start region and are re-zeroed.
                        for w in range(8):
                            m = nc.tensor.matmul(pa[0:1, 0:256],
                                                 wr[0:1, 0:1],
                                                 wr[0:1, 0:256],
                                                 start=True, stop=True)
                            _order(m, pe_tail)
                            pe_tail = m
                    if i == 0 and cp == 1:
                        pe_touch(wr[0:1, 768:770])  # wr mid-segment lane
                    pb, tb = new_mm_tile("pm1b")
                    for s in range(2):
                        mm(pa[:, s * 512:(s + 1) * 512],
                           wr[0:100, W1C + 2 * cp * 128:W1C + (2 * cp + 1) * 128],
                           rTs[s][:], start=True, stop=True)
                    for s in range(2):
                        mm(pb[:, s * 512:(s + 1) * 512],
                           wr[0:100, W1C + (2 * cp + 1) * 128:W1C + (2 * cp + 2) * 128],
                           rTs[s][:], start=True, stop=True)
                    act_touch(pb[0:1, 512:513])
                    silu(ha[:], pa[:], wf[:, B1C + 2 * cp:B1C + 2 * cp + 1],
                         ha, ta)
                    silu(hb[:], pb[:], wf[:, B1C + 2 * cp + 1:B1C + 2 * cp + 2],
                         hb, tb)
                    h1s.extend([ha, hb])
                    if cp == 0 and i + 1 < NCH:
                        emit_tr_half(i + 1, 0)
                    if cp == 1 and i + 1 < NCH:
                        emit_tr_half(i + 1, 1)
                    if cp == 2 and i >= 1:
                        emit_l4(i - 1)

                # L2: per pair, odd chamber shifted to rows 64:127
                h2s = []
                l2t = []
                for pr in range(3):
                    pm2, tg2 = new_mm_tile("pm2")
                    for s in range(2):
                        mm(pm2[:, s * 512:(s + 1) * 512],
                           wr[:, W2OC + pr * 128:W2OC + (pr + 1) * 128],
                           h1s[2 * pr + 1][:, s * 512:(s + 1) * 512],
                           start=True, stop=False)
                        mm(pm2[0:64, s * 512:(s + 1) * 512],
                           wr[:, W2EC + pr * 64:W2EC + (pr + 1) * 64],
                           h1s[2 * pr][:, s * 512:(s + 1) * 512],
                           start=False, stop=True)
                    l2t.append((pm2, tg2))
                for pr in range(3):
                    pm2, tg2 = l2t[pr]
                    if pr == 0:
                        act_touch(pm2[0:1, 512:513])
                    h2 = sbh.tile([128, T], F32R, tag="h2", bufs=4, name="h2")
                    silu(h2[:], pm2[:], wf[:, B2PC + pr:B2PC + pr + 1],
                         h2, tg2)
                    h2s.append(h2)

                # L3 pairs 0,1 merged into one tile; pair 2 separate
                h3a = sbh.tile([128, T], F32R, tag="h3a", bufs=2, name="h3a")
                pa3, ta3 = new_mm_tile("pm3")
                pe_touch(h2s[1][0:1, 0:2])   # newest rhs silu for pa3
                for s in range(2):
                    mm(pa3[:, s * 512:(s + 1) * 512],
                       wr[:, W3AC:W3AC + 128],
                       h2s[1][:, s * 512:(s + 1) * 512], start=True, stop=False)
                    mm(pa3[0:64, s * 512:(s + 1) * 512],
                       wr[:, W3BC0:W3BC0 + 64],
                       h2s[0][:, s * 512:(s + 1) * 512], start=False, stop=True)
                h3b = sbh.tile([64, T], F32R, tag="h3b", bufs=2, name="h3b")
                pc, tc_ = new_mm_tile("pmc")
                pe_touch(h2s[2][0:1, 0:2])   # newest rhs silu for pc
                for s in range(2):
                    mm(pc[0:64, s * 512:(s + 1) * 512],
                       wr[:, W3P2C:W3P2C + 64],
                       h2s[2][:, s * 512:(s + 1) * 512], start=True, stop=True)
                # silu(h3a) only needs pa3 (ready during silu(pm2_2)); keep
                # it ahead of the pc-dependent touch so ACT never idles here.
                act_touch(pa3[0:1, 512:513])
                silu(h3a[:], pa3[:], wf[:, B3PC:B3PC + 1], h3a, ta3)
                act_touch(pc[0:1, 512:513])
                silu(h3b[0:64, :], pc[0:64, :], wf[0:64, B3P2:B3P2 + 1],
                     h3b, tc_)
                h3_tiles[i] = (h3a, h3b)

            emit_l4(NCH - 1)

            # ---- coupled sigmoid recurrence on raw halves [96, 512] ----
            for s in range(2):
                act_touch(raw_sb[s][0:1, 0:1])  # assembly DMAs (one lane)
                sig = nc.scalar.activation(act_pp[s][1][:],
                                           raw_sb[s][0:96, :], AF.Sigmoid)
                _order(sig, act_tail)
                act_tail = sig
                nc.sync.dma_start(out=raw_d[:, s * 512:(s + 1) * 512],
                                  in_=raw_sb[s][:])
                dve_touch(raw_sb[s][0:1, 0:1])
                cpr = nc.vector.tensor_copy(raw_rh[s][:], raw_sb[s][:])
                _order(cpr, dve_tail)
                dve_tail = cpr
                pe_touch(raw_rh[s][0:1, 0:2])

            for kk in range(CF_ITERS):
                for s in range(2):
                    src = act_pp[s][(kk - 1) % 2]
                    dst = act_pp[s][kk % 2] if kk < CF_ITERS - 1 else act_oh[s]
                    pe_touch(src[0:1, 0:2])
                    pm5, tg5 = new_mm_tile("pm5", width=512)
                    mm(pm5[0:96, 0:512],
                       wr[0:96, CDC:CDC + 96],
                       src[:],
                       start=True, stop=False)
                    mm(pm5[0:96, 0:512],
                       wr[0:96, I96C:I96C + 96],
                       raw_rh[s][:],
                       start=False, stop=True)
                    act_touch(pm5[0:1, 0:1])
                    sg = nc.scalar.activation(dst[:], pm5[0:96, 0:512],
                                              AF.Sigmoid)
                    _order(sg, act_tail)
                    act_tail = sg
                    tag_state[tg5] = (dst, 0, 0)
                    if kk == CF_ITERS - 1:
                        nc.sync.dma_start(out=act_d[:, s * 512:(s + 1) * 512],
                                          in_=act_oh[s][:])

    return nc


def _pack_consts(W1, b1, W2, b2, W3, b3, W4, b4, coupling, decay):
    wf = np.zeros((128, FCOLS), dtype=np.float32)
    wf[:, IDC:IDC + 128] = np.eye(128, dtype=np.float32)
    for c in range(6):
        wf[:, B1C + c] = b1[c]
    for pr in range(3):
        wf[0:64, B2PC + pr] = b2[2 * pr]
        wf[64:128, B2PC + pr] = b2[2 * pr + 1]
    for c in range(4):
        wf[c * 32:(c + 1) * 32, B3PC] = b3[c]
    wf[0:32, B3P2] = b3[4]
    wf[32:64, B3P2] = b3[5]
    wf[0:96, B4C] = np.tile(b4, NCH)

    wr = np.zeros((128, RCOLS), dtype=np.float32)
    for c in range(6):
        wr[0:100, W1C + c * 128:W1C + (c + 1) * 128] = W1[c]
    for pr in range(3):
        wr[:, W2EC + pr * 64:W2EC + (pr + 1) * 64] = W2[2 * pr]
        wr[:, W2OC + pr * 128 + 64:W2OC + (pr + 1) * 128] = W2[2 * pr + 1]
    # L3 merged pairs 0,1: pair1 shifted to out rows 64:127
    wr[0:64, W3AC + 64:W3AC + 96] = W3[2]
    wr[64:128, W3AC + 96:W3AC + 128] = W3[3]
    wr[0:64, W3BC0:W3BC0 + 32] = W3[0]
    wr[64:128, W3BC0 + 32:W3BC0 + 64] = W3[1]
    # L3 pair 2: out rows 0:63
    wr[0:64, W3P2C:W3P2C + 32] = W3[4]
    wr[64:128, W3P2C + 32:W3P2C + 64] = W3[5]
    # L4: chambers 0-3 from h3a (rows 32c..), chambers 4,5 from h3b rows 0:64
    for c in range(4):
        wr[c * 32:(c + 1) * 32, W4AC + c] = W4[c]
    wr[0:32, W4BC + 4] = W4[4]
    wr[32:64, W4BC + 5] = W4[5]
    cd = (decay[:, None] * coupling * CF_K).astype(np.float32)
    for g in range(16):
        wr[6 * g:6 * g + 6, CDC + 6 * g:CDC + 6 * g + 6] = cd
    wr[0:96, I96C:I96C + 96] = np.eye(96, dtype=np.float32)
    return wf, wr


def _unshard(per_core, key):
    """[96, T] layout (row 6i+c, col j) -> [BS, 6] per core, concat.

    Col j = 512h + 128n + p of chunk i maps to sample i*T + 8p + 4h + n,
    except chunk 0 where the res DMA is split in sample halves and the
    mapping is 512h + 4p + n."""
    outs = []
    for r in per_core:
        a = r[key].reshape(NCH, 6, 2, 4, 128)      # [i, c, h, n, p]
        out = np.empty((BS, 6), dtype=a.dtype)
        out[0:T] = a[0].transpose(1, 3, 2, 0).reshape(T, 6)
        out[T:] = a[1:].transpose(0, 4, 2, 3, 1).reshape(BS - T, 6)
        outs.append(out)
    return np.concatenate(outs, axis=0)


def kernel(res, W1, b1, W2, b2, W3, b3, W4, b4, coupling, decay):
    res = np.asarray(res, dtype=np.float32)
    args = [np.asarray(a, dtype=np.float32)
            for a in (W1, b1, W2, b2, W3, b3, W4, b4, coupling, decay)]
    wf, wr = _pack_consts(*args)

    nc = build_module()
    in_maps = [
        {"res": np.ascontiguousarray(res[i * BS:(i + 1) * BS]),
         "wf": wf, "wr": wr}
        for i in range(NCORES)
    ]
    results = run_bass_kernel_spmd(nc, in_maps, core_ids=list(range(NCORES)))
    act = _unshard(results.results, "act_out")
    raw = _unshard(results.results, "raw_out")
    return act, raw
